# revision 24
# baseline (speedup 1.0000x reference)
"""Tensor-parallel multi-head attention for Trainium2 (8 NeuronCores).

v3: AllToAll-based output projection.

Problem: x:[2,16,2048,1024], wq/wk/wv:[64,1024], wo:[1024,1024]
  xq/xk/xv = einsum('bhsd,kd->bhsk', x, w)          (per-head, shared w)
  score    = xq @ xk.T / sqrt(1024); attn = softmax(score)
  out      = (attn @ xv) -> [B,S,H*dk] @ wo.T -> [B,S,1024]

Sharding: attention is head-parallel (2 heads x 2 batches = 4 pairs per
core, processed batch-interleaved: (h0,b0),(h0,b1),(h1,b0),(h1,b1)).
The output projection is TOKEN-parallel: core j computes all 1024
output dims for (b,s)-column slice [512j, 512j+512). The activation
redistribution is two AllToAll ops (one per head slot, 512KB each,
mesh algorithm) instead of per-pair AllGathers (which ran RDH at
~35us each and serialized on the single cc stream).

Attention pipeline per (pair, half): the ScalarE exp stream
(ACTIVATE [128,1024], ~1.11us) is kept back-to-back by emitting
scores(t+1) before attnv(t); v projection (col-tiled pairs),
vt transposes, and the next pair's q/k projection fill PE idle slots.
Softmax denominator via an all-ones column appended to V.
"""

import os
import sys

import numpy as np

sys.path.insert(0, "/opt/trn_rl_repo")

import ml_dtypes  # noqa: E402

import concourse.bass as bass  # noqa: E402
import concourse.mybir as mybir  # noqa: E402
import concourse.tile as tile  # noqa: E402
from concourse import bacc  # noqa: E402
from concourse.bass_utils import run_bass_kernel_spmd  # noqa: E402
from concourse.masks import make_identity  # noqa: E402

N_CORES = 8
B, H, S, D = 2, 16, 2048, 1024
DK = D // H            # 64
HPC = H // N_CORES     # heads per core = 2
PAIRS = B * HPC        # (b, h) pairs per core = 4
SC = 512               # s-chunk (PSUM free-dim limit for f32)
NSC = S // SC          # 4 s-chunks per pair
NT = S // 128          # 16 t-tiles
NDC = D // 128         # 8 contraction chunks of 128
BS = B * S             # 4096 flattened (b, s) columns
INV_SCALE = 1.0 / 32.0  # 1/sqrt(D)

F32 = mybir.dt.float32
BF16 = mybir.dt.bfloat16

_GRAPH = None
LAST_RESULTS = None  # BassKernelResults of the most recent run (for test.py)


def _build_graph():
    nc = bacc.Bacc("TRN2", target_bir_lowering=False, num_devices=N_CORES)

    # pairs in processing order q: (head-slot hl, batch b) = divmod(q, 2)
    xt = nc.declare_dram_parameter("xt", [PAIRS, D, S], BF16, isOutput=False)
    wqk = nc.declare_dram_parameter("wqk", [D, 128], BF16, isOutput=False)
    wv2 = nc.declare_dram_parameter("wv2", [D, 128], BF16, isOutput=False)
    wo = nc.declare_dram_parameter("wo", [D, D], BF16, isOutput=False)
    out = nc.declare_dram_parameter("out", [NDC, 128, SC], F32, isOutput=True)

    Exp = mybir.ActivationFunctionType.Exp

    with tile.TileContext(nc) as tc:
        with (
            tc.tile_pool(name="const", bufs=1) as cpool,
            tc.tile_pool(name="dram", bufs=1, space="DRAM") as dpool,
            tc.tile_pool(name="xin", bufs=2) as xpool,
            tc.tile_pool(name="qkv", bufs=2) as qkvpool,
            tc.tile_pool(name="vtiles", bufs=2) as vpool,
            tc.tile_pool(name="exp", bufs=3) as epool,
            tc.tile_pool(name="norm", bufs=2) as npool,
            tc.tile_pool(name="aio", bufs=1) as apool,
            tc.tile_pool(name="oout", bufs=2) as opool,
            tc.tile_pool(name="ps_proj", bufs=2, space="PSUM") as ps_proj,
            tc.tile_pool(name="ps_sc", bufs=2, space="PSUM") as ps_sc,
            tc.tile_pool(name="ps_ou", bufs=1, space="PSUM") as ps_ou,
        ):
            # Weights, bf16, laid out [128 partitions, chunk, m]
            wqk_sb = cpool.tile([128, NDC, 128], BF16)
            nc.sync.dma_start(
                out=wqk_sb[:], in_=wqk[:].rearrange("(c p) m -> p c m", p=128)
            )
            wv2_sb = cpool.tile([128, NDC, 128], BF16)
            nc.sync.dma_start(
                out=wv2_sb[:], in_=wv2[:].rearrange("(c p) m -> p c m", p=128)
            )
            # full output-projection weight: [128, c-chunk, m-tile, 128]
            # (loaded mid-kernel: 2MB would delay the x ramp)
            wo_sb = cpool.tile([128, NDC, NDC, 128], BF16)

            def load_wo():
                nc.sync.dma_start(
                    out=wo_sb[:],
                    in_=wo[:].rearrange(
                        "(c p) (m w) -> p c m w", p=128, w=128
                    ),
                )

            ident64 = cpool.tile([64, 64], BF16)
            make_identity(nc, ident64[:])

            # AllToAll bounce buffers: two ops, one per head slot.
            # in[j] = this core's activations for dest core j's token slice.
            a2a_in = [
                dpool.tile([N_CORES, DK, SC], BF16, name=f"a2a_in{g}")
                for g in range(2)
            ]
            a2a_out = [
                dpool.tile([N_CORES, DK, SC], BF16, name=f"a2a_out{g}")
                for g in range(2)
            ]
            warm_in = dpool.tile([N_CORES, 8, 16], BF16)
            warm_out = dpool.tile([N_CORES, 8, 16], BF16, name="warm_out")

            # Warmup collective: triggered as early as possible so the
            # one-time collective-runtime init overlaps the ramp.
            nc.vector.memset(
                warm_in_sb0 := cpool.tile([8, 8 * 16], BF16, name="warm_sb"),
                0.0,
            )
            nc.sync.dma_start(
                out=warm_in[:],
                in_=warm_in_sb0[:].rearrange("a (c m) -> a c m", c=8),
            )
            nc.gpsimd.collective_compute(
                "AllToAll",
                mybir.AluOpType.bypass,
                replica_groups=[list(range(N_CORES))],
                ins=[warm_in.opt()],
                outs=[warm_out.opt()],
            )

            # gathered activations for my token slice: [128, c, 512]
            # rows 0:64 of chunk c = head 2c (op 0), 64:128 = head 2c+1 (op 1)
            asb = apool.tile([128, NDC, SC], BF16, name="asb")

            def emit_xT(q):
                # n-major sub-block loads: early chunks land after ~1MB
                xT = xpool.tile([128, NDC, S], BF16, tag="xT", name=f"xT{q}")
                for n in range(NSC):
                    for c in range(NDC):
                        nc.sync.dma_start(
                            out=xT[:, c, n * SC : (n + 1) * SC],
                            in_=xt[q][
                                c * 128 : (c + 1) * 128, n * SC : (n + 1) * SC
                            ],
                        )
                return xT

            def alloc_qk(q):
                # qk: partitions 0:64 = q, 64:128 = k
                # qk2: partitions 0:64 = k, 64:128 = q (for strip alternation)
                qk_sb = qkvpool.tile([128, S], BF16, tag="qk", name=f"qk{q}")
                qk2_sb = qkvpool.tile([128, S], BF16, tag="qk2", name=f"qk2{q}")
                return qk_sb, qk2_sb

            qk_ps = {}

            def emit_qk(xT, qk_sb, qk2_sb, n, cr=tuple(range(NDC))):
                """c-chunks cr of one n-chunk of the q/k projection
                (+ duplication after the last chunk)."""
                nsl = slice(n * SC, (n + 1) * SC)
                if cr[0] == 0:
                    qk_ps[n] = ps_proj.tile(
                        [128, SC], F32, tag="proj_ps", name="ps_qk"
                    )
                ps_qk = qk_ps[n]
                for c in cr:
                    nc.tensor.matmul(
                        ps_qk[:],
                        wqk_sb[:, c, :],
                        xT[:, c, nsl],
                        start=(c == 0),
                        stop=(c == NDC - 1),
                    )
                if cr[-1] == NDC - 1:
                    nc.vector.tensor_copy(qk_sb[:, nsl], ps_qk[:])
                    nc.vector.tensor_copy(qk2_sb[0:64, nsl], ps_qk[64:128, :])
                    nc.vector.tensor_copy(qk2_sb[64:128, nsl], ps_qk[0:64, :])
                    del qk_ps[n]

            def alloc_v(q):
                vT_sb = qkvpool.tile([64, S], BF16, tag="vT")
                vt = [
                    vpool.tile([128, 65], BF16, tag=f"vt{t}", name=f"vt{t}")
                    for t in range(NT)
                ]
                return vT_sb, vt

            v_ps = {}

            def v_mm(xT, vT_sb, jj, cr=tuple(range(NDC))):
                """v projection for chunk pair (2jj, 2jj+1), col-tiled so
                the two chunks stream concurrently in the array halves."""
                sla = slice(2 * jj * SC, (2 * jj + 1) * SC)
                slb = slice((2 * jj + 1) * SC, (2 * jj + 2) * SC)
                if cr[0] == 0:
                    v_ps[jj] = ps_proj.tile(
                        [128, SC], F32, tag="proj_ps", name="ps_v"
                    )
                ps_v = v_ps[jj]
                for c in cr:
                    nc.tensor.matmul(
                        ps_v[0:64, :],
                        wv2_sb[:, c, 0:64],
                        xT[:, c, sla],
                        start=(c == 0),
                        stop=(c == NDC - 1),
                        tile_position=(0, 0),
                        skip_group_check=True,
                    )
                    nc.tensor.matmul(
                        ps_v[64:128, :],
                        wv2_sb[:, c, 64:128],
                        xT[:, c, slb],
                        start=(c == 0),
                        stop=(c == NDC - 1),
                        tile_position=(0, 64),
                        skip_group_check=True,
                    )
                if cr[-1] == NDC - 1:
                    nc.vector.tensor_copy(vT_sb[:, sla], ps_v[0:64, :])
                    nc.vector.tensor_copy(vT_sb[:, slb], ps_v[64:128, :])
                    del v_ps[jj]

            def v_tp(vT_sb, vt, t0, ntp=2):
                """PE transposes for vt tiles t0..t0+ntp (+ ones column)."""
                vt_q = ps_proj.tile(
                    [128, ntp, 64], BF16, tag="proj_ps", name="vt_q"
                )
                for tt in range(t0, t0 + ntp):
                    nc.tensor.transpose(
                        vt_q[:, tt - t0, :],
                        vT_sb[:, tt * 128 : (tt + 1) * 128],
                        ident64[:],
                    )
                    nc.vector.memset(vt[tt][:, 64:65], 1.0)
                    nc.vector.tensor_copy(
                        vt[tt][:, 0:64], vt_q[:, tt - t0, :]
                    )

            def emit_scores(qk_sb, qk2_sb, half, t):
                """Score matmuls for t-tile t, both n-chunks of this half,
                on alternating 64-row strips (concurrent execution)."""
                tsl = slice(t * 128, (t + 1) * 128)
                sc = ps_sc.tile([128, 2 * SC], F32, tag="sc", name="sc")
                for i in range(2):
                    n = 2 * half + i
                    nsl = slice(n * SC, (n + 1) * SC)
                    if n % 2 == 0:
                        nc.tensor.matmul(
                            sc[:, i * SC : (i + 1) * SC],
                            qk2_sb[0:64, tsl],
                            qk_sb[0:64, nsl],
                            start=True,
                            stop=True,
                            tile_position=(0, 0),
                        )
                    else:
                        nc.tensor.matmul(
                            sc[:, i * SC : (i + 1) * SC],
                            qk_sb[64:128, tsl],
                            qk2_sb[64:128, nsl],
                            start=True,
                            stop=True,
                            tile_position=(64, 0),
                        )
                return sc

            def run_half(q, half, qk_sb, qk2_sb, vt, fillers, sc0):
                """Software-pipelined t-loop: ACT(t) and scores(t+1) are
                emitted before attnv(t) so the exp stream never waits on
                the attn@v matmuls."""
                b = q % 2
                ascope = nc.named_scope(f"attn{q}h{half}")
                ascope.__enter__()
                ou = [
                    ps_ou.tile([65, SC], F32, tag=f"ou{i}", name=f"ou{i}")
                    for i in range(2)
                ]
                sc_cur = sc0
                last_mm = None
                for t in range(NT):
                    pexp = epool.tile([128, 2 * SC], BF16, tag="pexp")
                    nc.scalar.activation(
                        pexp[:], sc_cur[:], Exp, scale=INV_SCALE
                    )
                    if t + 1 < NT:
                        sc_cur = emit_scores(qk_sb, qk2_sb, half, t + 1)
                    for f in fillers.get(t, ()):
                        f()
                    for i in range(2):
                        last_mm = nc.tensor.matmul(
                            ou[i][:],
                            vt[t][:],
                            pexp[:, i * SC : (i + 1) * SC],
                            start=(t == 0),
                            stop=(t == NT - 1),
                        )
                # normalize and ship the two finished n-chunks to their
                # destination cores' slots of this head-slot's AllToAll
                for i in range(2):
                    n = 2 * half + i
                    ou_sb = npool.tile([65, SC], F32, tag="ou_sb")
                    nc.vector.tensor_copy(ou_sb[:], ou[i][:])
                    den0 = npool.tile([1, SC], F32, tag="den0")
                    nc.vector.tensor_copy(den0[0:1, :], ou_sb[64:65, :])
                    recip = npool.tile([1, SC], F32, tag="recip")
                    nc.vector.reciprocal_approx_fast(
                        recip[0:1, :], den0[0:1, :]
                    )
                    bcast = npool.tile([64, SC], F32, tag="bcast")
                    nc.gpsimd.partition_broadcast(bcast[:], recip[0:1, :])
                    onorm = npool.tile([64, SC], BF16, tag="onorm")
                    nc.vector.tensor_mul(
                        onorm[:], ou_sb[0:64, :], bcast[:]
                    )
                    nc.sync.dma_start(
                        out=a2a_in[q // 2][4 * b + n][:, :],
                        in_=onorm[:],
                    )
                ascope.__exit__(None, None, None)
                return last_mm

            # ~3.5us of dummy matmuls during the initial DMA window so the
            # PE clock gate (HAM) is already open when real work issues.
            warm_ps = ps_proj.tile([64, SC], F32, tag="proj_ps", name="warm_ps")
            for w in range(8):
                nc.tensor.matmul(
                    warm_ps[:],
                    wqk_sb[:, 0, 0:64],
                    wqk_sb[:, 4 * (w % 2) : 4 * (w % 2) + 4, :],
                    start=True,
                    stop=True,
                )

            # ---- pair-0 prologue ----
            xT_t = {0: emit_xT(0)}
            qk_t = {0: alloc_qk(0)}
            v_t = {0: alloc_v(0)}
            emit_qk(xT_t[0], *qk_t[0], 0)
            emit_qk(xT_t[0], *qk_t[0], 1)

            last_mm = None
            for q in range(PAIRS):
                xT = xT_t.pop(q)
                qk_sb, qk2_sb = qk_t.pop(q)
                vT_sb, vt = v_t.pop(q)

                # boundary: first scores, then this pair's first v half
                sc0 = emit_scores(qk_sb, qk2_sb, 0, 0)
                v_mm(xT, vT_sb, 0)

                # NOTE: vt[t]'s writes must be EMITTED before attnv(t)
                # (iteration t) or Tile resolves the read against the
                # previous pair's tile -- so v_tp(t0) sits at slot <= t0.
                CLO, CHI = tuple(range(NDC // 2)), tuple(range(NDC // 2, NDC))
                f_h0 = {
                    0: [lambda: v_tp(vT_sb, vt, 0)],
                    1: [lambda: v_tp(vT_sb, vt, 2)],
                    2: [lambda: v_mm(xT, vT_sb, 1, CLO)],
                    3: [lambda: v_tp(vT_sb, vt, 4)],
                    4: [lambda: v_tp(vT_sb, vt, 6)],
                    5: [lambda: v_mm(xT, vT_sb, 1, CHI)],
                    6: [lambda: v_tp(vT_sb, vt, 8)],
                    7: [lambda: v_tp(vT_sb, vt, 10)],
                    8: [lambda: v_tp(vT_sb, vt, 12)],
                    9: [lambda: v_tp(vT_sb, vt, 14)],
                }
                if q + 1 < PAIRS:
                    f_h0[3].insert(
                        0, lambda: xT_t.__setitem__(q + 1, emit_xT(q + 1))
                    )
                    qk_t[q + 1] = alloc_qk(q + 1)
                    v_t[q + 1] = alloc_v(q + 1)
                    qkn, qk2n = qk_t[q + 1]
                    if q > 0:
                        f_h0[12] = [
                            lambda: emit_qk(xT_t[q + 1], qkn, qk2n, 0, CLO)
                        ]
                        f_h0[13] = [
                            lambda: emit_qk(xT_t[q + 1], qkn, qk2n, 0, CHI)
                        ]
                        f_h0[14] = [
                            lambda: emit_qk(xT_t[q + 1], qkn, qk2n, 1, CLO)
                        ]
                        f_h0[15] = [
                            lambda: emit_qk(xT_t[q + 1], qkn, qk2n, 1, CHI)
                        ]
                if q == 0:
                    # pair 0's own chunks 2/3 have in-half deadlines
                    # (kd tile t needs chunk t//4 by iteration t-1)
                    f_h0[2].append(lambda: emit_qk(xT, qk_sb, qk2_sb, 2, CLO))
                    f_h0[4].append(lambda: emit_qk(xT, qk_sb, qk2_sb, 2, CHI))
                    f_h0[6].append(lambda: emit_qk(xT, qk_sb, qk2_sb, 3, CLO))
                    f_h0[7].append(lambda: emit_qk(xT, qk_sb, qk2_sb, 3, CHI))
                if q == 1:
                    f_h0[10] = [load_wo]
                run_half(q, 0, qk_sb, qk2_sb, vt, f_h0, sc0)

                sc0 = emit_scores(qk_sb, qk2_sb, 1, 0)
                f_h1 = {}
                if q + 1 < PAIRS:
                    qkn, qk2n = qk_t[q + 1]
                    for k in range(4):
                        n, cr = 2 + k // 2, (CLO if k % 2 == 0 else CHI)
                        f_h1[2 * k] = [
                            lambda n=n, cr=cr: emit_qk(
                                xT_t[q + 1], qkn, qk2n, n, cr
                            )
                        ]
                    if q == 0:
                        # pair-1 chunks 0/1 didn't fit in pair-0's h0
                        for k in range(4):
                            n, cr = k // 2, (CLO if k % 2 == 0 else CHI)
                            f_h1[9 + k] = [
                                lambda n=n, cr=cr: emit_qk(
                                    xT_t[q + 1], qkn, qk2n, n, cr
                                )
                            ]
                last_mm = run_half(q, 1, qk_sb, qk2_sb, vt, f_h1, sc0)

                if q == 1:
                    # head-slot 0 of every core is done: redistribute
                    nc.gpsimd.collective_compute(
                        "AllToAll",
                        mybir.AluOpType.bypass,
                        replica_groups=[list(range(N_CORES))],
                        ins=[a2a_in[0].opt()],
                        outs=[a2a_out[0].opt()],
                    )
                    # even-head rows of the gathered activations
                    for c in range(NDC):
                        nc.sync.dma_start(
                            out=asb[0:64, c, :], in_=a2a_out[0][c][:, :]
                        )

            # ---- head-slot-1 redistribution + output projection tail ----
            nc.gpsimd.collective_compute(
                "AllToAll",
                mybir.AluOpType.bypass,
                replica_groups=[list(range(N_CORES))],
                ins=[a2a_in[1].opt()],
                outs=[a2a_out[1].opt()],
            )
            # keep the PE clock gate open across the AllToAll wait
            for w in range(30):
                nc.tensor.matmul(
                    warm_ps[:],
                    wqk_sb[:, 0, 0:64],
                    wqk_sb[:, 4 * (w % 2) : 4 * (w % 2) + 4, :],
                    start=True,
                    stop=True,
                )

            oscope = nc.named_scope("outproj")
            oscope.__enter__()
            # 6 m-tile accumulators run c-outer so the first matmuls start
            # as soon as the first odd-row chunk lands, pipelined with the
            # remaining asb DMAs; m=6,7 run m-outer afterwards.
            o_ps6 = [
                ps_sc.tile([128, SC], F32, tag="sc", name=f"o_ps{m}")
                for m in range(2)
            ] + [
                ps_proj.tile([128, SC], F32, tag="proj_ps", name=f"o_ps{m+2}")
                for m in range(2)
            ] + [
                ps_ou.tile([128, SC], F32, tag=f"ou{m}", name=f"o_ps{m+4}")
                for m in range(2)
            ]
            for c in range(NDC):
                nc.sync.dma_start(
                    out=asb[64:128, c, :], in_=a2a_out[1][c][:, :]
                )
                for m in range(6):
                    mm = nc.tensor.matmul(
                        o_ps6[m][:],
                        wo_sb[:, c, m, :],
                        asb[:, c, :],
                        start=(c == 0),
                        stop=(c == NDC - 1),
                    )
                    if last_mm is not None:
                        tile.add_dep_helper(
                            mm.ins, last_mm.ins, sync=False,
                            reason="outproj after attention",
                        )
                        last_mm = None
            for m in range(6):
                o_sb = opool.tile([128, SC], F32, tag="o_sb")
                nc.vector.tensor_copy(o_sb[:], o_ps6[m][:])
                nc.sync.dma_start(out=out[m][:, :], in_=o_sb[:])
            for m in (6, 7):
                o_ps = ps_sc.tile([128, SC], F32, tag="sc", name=f"o_ps{m}")
                for c in range(NDC):
                    nc.tensor.matmul(
                        o_ps[:],
                        wo_sb[:, c, m, :],
                        asb[:, c, :],
                        start=(c == 0),
                        stop=(c == NDC - 1),
                    )
                o_sb = opool.tile([128, SC], F32, tag="o_sb")
                nc.vector.tensor_copy(o_sb[:], o_ps[:])
                nc.sync.dma_start(out=out[m][:, :], in_=o_sb[:])
            oscope.__exit__(None, None, None)

    return nc


def _get_graph():
    global _GRAPH
    if _GRAPH is None:
        _GRAPH = _build_graph()
        if not _GRAPH.is_finalized():
            _GRAPH.finalize()
    return _GRAPH


def assemble(outs):
    # outs[r]: [8, 128, 512] f32 = out.T[:, 512r : 512r+512]
    full_t = np.concatenate(
        [np.asarray(o).reshape(D, SC) for o in outs], axis=1
    )  # [D, B*S]
    return np.ascontiguousarray(full_t.T).reshape(B, S, D)


def kernel(x, wq, wk, wv, wo):
    global LAST_RESULTS
    x = np.asarray(x, dtype=np.float32)
    wq = np.asarray(wq, dtype=np.float32)
    wk = np.asarray(wk, dtype=np.float32)
    wv = np.asarray(wv, dtype=np.float32)
    wo = np.asarray(wo, dtype=np.float32)

    bf16 = ml_dtypes.bfloat16
    # x transposed to [B, H, D, S] once (feeds matmuls as the moving operand)
    xt_all = np.ascontiguousarray(x.transpose(0, 1, 3, 2)).astype(bf16)
    wqk_t = np.ascontiguousarray(
        np.concatenate([wq, wk], axis=0).T
    ).astype(bf16)  # [D, 128]
    wv_t = np.ascontiguousarray(wv.T).astype(bf16)  # [D, 64]
    wv2_t = np.ascontiguousarray(
        np.concatenate([wv_t, wv_t], axis=1)
    )  # [D, 128] duplicated for col-tiled v projection
    wo_t = np.ascontiguousarray(wo.T).astype(bf16)  # [D, D], full per core

    in_maps = []
    for r in range(N_CORES):
        h0 = HPC * r
        # pair order: q = hl*B + b -> (b, h0+hl)
        xt_np = np.ascontiguousarray(
            xt_all[:, h0 : h0 + HPC]
            .transpose(1, 0, 2, 3)
            .reshape(PAIRS, D, S)
        )
        in_maps.append(
            {"xt": xt_np, "wqk": wqk_t, "wv2": wv2_t, "wo": wo_t}
        )

    nc = _get_graph()
    trace = bool(os.environ.get("BASS_TRACE"))
    if trace:
        try:  # tracing needs the axon NTFF hook; fall back cleanly
            from antenv.axon_hooks import get_axon_ntff_profile_hook  # noqa: F401
        except ImportError:
            trace = False
    tk = {}
    tc_env = os.environ.get("TRACE_CORES")
    if tc_env:
        tk["trace_cores"] = [int(c) for c in tc_env.split(",")]
    LAST_RESULTS = run_bass_kernel_spmd(
        nc, in_maps, core_ids=list(range(N_CORES)), trace=trace, **tk
    )
    outs = [LAST_RESULTS.results[r]["out"] for r in range(N_CORES)]
    return assemble(outs)


# revision 31
# speedup vs baseline: 1.0336x; 1.0336x over previous
"""Tensor-parallel multi-head attention for Trainium2 (8 NeuronCores).

v3: AllToAll-based output projection.

Problem: x:[2,16,2048,1024], wq/wk/wv:[64,1024], wo:[1024,1024]
  xq/xk/xv = einsum('bhsd,kd->bhsk', x, w)          (per-head, shared w)
  score    = xq @ xk.T / sqrt(1024); attn = softmax(score)
  out      = (attn @ xv) -> [B,S,H*dk] @ wo.T -> [B,S,1024]

Sharding: attention is head-parallel (2 heads x 2 batches = 4 pairs per
core, processed batch-interleaved: (h0,b0),(h0,b1),(h1,b0),(h1,b1)).
The output projection is TOKEN-parallel: core j computes all 1024
output dims for (b,s)-column slice [512j, 512j+512). The activation
redistribution is two AllToAll ops (one per head slot, 512KB each,
mesh algorithm) instead of per-pair AllGathers (which ran RDH at
~35us each and serialized on the single cc stream).

Attention pipeline per (pair, half): the ScalarE exp stream
(ACTIVATE [128,1024], ~1.11us) is kept back-to-back by emitting
scores(t+1) before attnv(t); v projection (col-tiled pairs),
vt transposes, and the next pair's q/k projection fill PE idle slots.
Softmax denominator via an all-ones column appended to V.
"""

import os
import sys

import numpy as np

sys.path.insert(0, "/opt/trn_rl_repo")

import ml_dtypes  # noqa: E402

import concourse.bass as bass  # noqa: E402
import concourse.mybir as mybir  # noqa: E402
import concourse.tile as tile  # noqa: E402
from concourse import bacc  # noqa: E402
from concourse.bass_utils import run_bass_kernel_spmd  # noqa: E402
from concourse.masks import make_identity  # noqa: E402

N_CORES = 8
B, H, S, D = 2, 16, 2048, 1024
DK = D // H            # 64
HPC = H // N_CORES     # heads per core = 2
PAIRS = B * HPC        # (b, h) pairs per core = 4
SC = 512               # s-chunk (PSUM free-dim limit for f32)
NSC = S // SC          # 4 s-chunks per pair
NT = S // 128          # 16 t-tiles
NDC = D // 128         # 8 contraction chunks of 128
BS = B * S             # 4096 flattened (b, s) columns
INV_SCALE = 1.0 / 32.0  # 1/sqrt(D)

F32 = mybir.dt.float32
BF16 = mybir.dt.bfloat16

_GRAPH = None
LAST_RESULTS = None  # BassKernelResults of the most recent run (for test.py)


def _build_graph():
    nc = bacc.Bacc("TRN2", target_bir_lowering=False, num_devices=N_CORES)

    # pairs in processing order q: (head-slot hl, batch b) = divmod(q, 2)
    xt = nc.declare_dram_parameter("xt", [PAIRS, D, S], BF16, isOutput=False)
    wqk = nc.declare_dram_parameter("wqk", [D, 128], BF16, isOutput=False)
    wv2 = nc.declare_dram_parameter("wv2", [D, 128], BF16, isOutput=False)
    wo = nc.declare_dram_parameter("wo", [D, D], BF16, isOutput=False)
    out = nc.declare_dram_parameter("out", [NDC, 128, SC], F32, isOutput=True)

    Exp = mybir.ActivationFunctionType.Exp

    with tile.TileContext(nc) as tc:
        with (
            tc.tile_pool(name="const", bufs=1) as cpool,
            tc.tile_pool(name="dram", bufs=1, space="DRAM") as dpool,
            tc.tile_pool(name="xin", bufs=2) as xpool,
            tc.tile_pool(name="qkv", bufs=2) as qkvpool,
            tc.tile_pool(name="vtiles", bufs=2) as vpool,
            tc.tile_pool(name="exp", bufs=3) as epool,
            tc.tile_pool(name="norm", bufs=2) as npool,
            tc.tile_pool(name="aio", bufs=1) as apool,
            tc.tile_pool(name="oout", bufs=2) as opool,
            tc.tile_pool(name="ps_proj", bufs=2, space="PSUM") as ps_proj,
            tc.tile_pool(name="ps_sc", bufs=2, space="PSUM") as ps_sc,
            tc.tile_pool(name="ps_ou", bufs=1, space="PSUM") as ps_ou,
        ):
            # Weights, bf16, laid out [128 partitions, chunk, m]
            wqk_sb = cpool.tile([128, NDC, 128], BF16)
            nc.sync.dma_start(
                out=wqk_sb[:], in_=wqk[:].rearrange("(c p) m -> p c m", p=128)
            )
            wv2_sb = cpool.tile([128, NDC, 128], BF16)
            nc.sync.dma_start(
                out=wv2_sb[:], in_=wv2[:].rearrange("(c p) m -> p c m", p=128)
            )
            # full output-projection weight: [128, c-chunk, m-tile, 128]
            # (loaded mid-kernel: 2MB would delay the x ramp)
            wo_sb = cpool.tile([128, NDC, NDC, 128], BF16)

            def load_wo():
                nc.sync.dma_start(
                    out=wo_sb[:],
                    in_=wo[:].rearrange(
                        "(c p) (m w) -> p c m w", p=128, w=128
                    ),
                )

            ident64 = cpool.tile([64, 64], BF16)
            make_identity(nc, ident64[:])

            # AllToAll bounce buffers: two ops, one per head slot.
            # in[j] = this core's activations for dest core j's token slice.
            a2a_in = [
                dpool.tile([N_CORES, DK, SC], BF16, name=f"a2a_in{g}")
                for g in range(2)
            ]
            a2a_out = [
                dpool.tile([N_CORES, DK, SC], BF16, name=f"a2a_out{g}")
                for g in range(2)
            ]
            warm_in = dpool.tile([N_CORES, 8, 16], BF16)
            warm_out = dpool.tile([N_CORES, 8, 16], BF16, name="warm_out")

            # Warmup collective: triggered as early as possible so the
            # one-time collective-runtime init overlaps the ramp.
            nc.vector.memset(
                warm_in_sb0 := cpool.tile([8, 8 * 16], BF16, name="warm_sb"),
                0.0,
            )
            nc.sync.dma_start(
                out=warm_in[:],
                in_=warm_in_sb0[:].rearrange("a (c m) -> a c m", c=8),
            )
            nc.gpsimd.collective_compute(
                "AllToAll",
                mybir.AluOpType.bypass,
                replica_groups=[list(range(N_CORES))],
                ins=[warm_in.opt()],
                outs=[warm_out.opt()],
            )

            # gathered activations for my token slice: [128, c, 512]
            # rows 0:64 of chunk c = head 2c (op 0), 64:128 = head 2c+1 (op 1)
            asb = apool.tile([128, NDC, SC], BF16, name="asb")

            def emit_xT(q):
                # n-major sub-block loads: early chunks land after ~1MB
                xT = xpool.tile([128, NDC, S], BF16, tag="xT", name=f"xT{q}")
                for n in range(NSC):
                    for c in range(NDC):
                        nc.sync.dma_start(
                            out=xT[:, c, n * SC : (n + 1) * SC],
                            in_=xt[q][
                                c * 128 : (c + 1) * 128, n * SC : (n + 1) * SC
                            ],
                        )
                return xT

            def alloc_qk(q):
                # qk: partitions 0:64 = q, 64:128 = k
                # qk2: partitions 0:64 = k, 64:128 = q (for strip alternation)
                qk_sb = qkvpool.tile([128, S], BF16, tag="qk", name=f"qk{q}")
                qk2_sb = qkvpool.tile([128, S], BF16, tag="qk2", name=f"qk2{q}")
                return qk_sb, qk2_sb

            qk_ps = {}

            def emit_qk(xT, qk_sb, qk2_sb, n, cr=tuple(range(NDC))):
                """c-chunks cr of one n-chunk of the q/k projection
                (+ duplication after the last chunk)."""
                nsl = slice(n * SC, (n + 1) * SC)
                if cr[0] == 0:
                    qk_ps[n] = ps_proj.tile(
                        [128, SC], F32, tag="proj_ps", name="ps_qk"
                    )
                ps_qk = qk_ps[n]
                for c in cr:
                    nc.tensor.matmul(
                        ps_qk[:],
                        wqk_sb[:, c, :],
                        xT[:, c, nsl],
                        start=(c == 0),
                        stop=(c == NDC - 1),
                    )
                if cr[-1] == NDC - 1:
                    nc.vector.tensor_copy(qk_sb[:, nsl], ps_qk[:])
                    nc.vector.tensor_copy(qk2_sb[0:64, nsl], ps_qk[64:128, :])
                    nc.vector.tensor_copy(qk2_sb[64:128, nsl], ps_qk[0:64, :])
                    del qk_ps[n]

            def alloc_v(q):
                vT_sb = qkvpool.tile([64, S], BF16, tag="vT")
                vt = [
                    vpool.tile([128, 65], BF16, tag=f"vt{t}", name=f"vt{t}")
                    for t in range(NT)
                ]
                return vT_sb, vt

            v_ps = {}

            def v_mm(xT, vT_sb, jj, cr=tuple(range(NDC))):
                """v projection for chunk pair (2jj, 2jj+1), col-tiled so
                the two chunks stream concurrently in the array halves."""
                sla = slice(2 * jj * SC, (2 * jj + 1) * SC)
                slb = slice((2 * jj + 1) * SC, (2 * jj + 2) * SC)
                if cr[0] == 0:
                    v_ps[jj] = ps_proj.tile(
                        [128, SC], F32, tag="proj_ps", name="ps_v"
                    )
                ps_v = v_ps[jj]
                for c in cr:
                    nc.tensor.matmul(
                        ps_v[0:64, :],
                        wv2_sb[:, c, 0:64],
                        xT[:, c, sla],
                        start=(c == 0),
                        stop=(c == NDC - 1),
                        tile_position=(0, 0),
                        skip_group_check=True,
                    )
                    nc.tensor.matmul(
                        ps_v[64:128, :],
                        wv2_sb[:, c, 64:128],
                        xT[:, c, slb],
                        start=(c == 0),
                        stop=(c == NDC - 1),
                        tile_position=(0, 64),
                        skip_group_check=True,
                    )
                if cr[-1] == NDC - 1:
                    nc.vector.tensor_copy(vT_sb[:, sla], ps_v[0:64, :])
                    nc.vector.tensor_copy(vT_sb[:, slb], ps_v[64:128, :])
                    del v_ps[jj]

            def v_tp(vT_sb, vt, t0, ntp=2):
                """PE transposes for vt tiles t0..t0+ntp (+ ones column)."""
                vt_q = ps_proj.tile(
                    [128, ntp, 64], BF16, tag="proj_ps", name="vt_q"
                )
                for tt in range(t0, t0 + ntp):
                    nc.tensor.transpose(
                        vt_q[:, tt - t0, :],
                        vT_sb[:, tt * 128 : (tt + 1) * 128],
                        ident64[:],
                    )
                    nc.vector.memset(vt[tt][:, 64:65], 1.0)
                    nc.vector.tensor_copy(
                        vt[tt][:, 0:64], vt_q[:, tt - t0, :]
                    )

            def emit_scores(qk_sb, qk2_sb, half, t):
                """Score matmuls for t-tile t, both n-chunks of this half,
                on alternating 64-row strips (concurrent execution)."""
                tsl = slice(t * 128, (t + 1) * 128)
                sc = ps_sc.tile([128, 2 * SC], F32, tag="sc", name="sc")
                for i in range(2):
                    n = 2 * half + i
                    nsl = slice(n * SC, (n + 1) * SC)
                    if n % 2 == 0:
                        nc.tensor.matmul(
                            sc[:, i * SC : (i + 1) * SC],
                            qk2_sb[0:64, tsl],
                            qk_sb[0:64, nsl],
                            start=True,
                            stop=True,
                            tile_position=(0, 0),
                        )
                    else:
                        nc.tensor.matmul(
                            sc[:, i * SC : (i + 1) * SC],
                            qk_sb[64:128, tsl],
                            qk2_sb[64:128, nsl],
                            start=True,
                            stop=True,
                            tile_position=(64, 0),
                        )
                return sc

            def run_half(q, half, qk_sb, qk2_sb, vt, fillers, sc0):
                """Software-pipelined t-loop: ACT(t) and scores(t+1) are
                emitted before attnv(t) so the exp stream never waits on
                the attn@v matmuls."""
                b = q % 2
                ascope = nc.named_scope(f"attn{q}h{half}")
                ascope.__enter__()
                ou = [
                    ps_ou.tile([65, SC], F32, tag=f"ou{i}", name=f"ou{i}")
                    for i in range(2)
                ]
                sc_cur = sc0
                last_mm = None
                for t in range(NT):
                    pexp = epool.tile([128, 2 * SC], BF16, tag="pexp")
                    nc.scalar.activation(
                        pexp[:], sc_cur[:], Exp, scale=INV_SCALE
                    )
                    if t + 1 < NT:
                        sc_cur = emit_scores(qk_sb, qk2_sb, half, t + 1)
                    for f in fillers.get(t, ()):
                        f()
                    for i in range(2):
                        last_mm = nc.tensor.matmul(
                            ou[i][:],
                            vt[t][:],
                            pexp[:, i * SC : (i + 1) * SC],
                            start=(t == 0),
                            stop=(t == NT - 1),
                        )
                # normalize and ship the two finished n-chunks to their
                # destination cores' slots of this head-slot's AllToAll;
                # stages interleaved across the two chunks so the DVE/GpSimd
                # chain to the last DMA (the collective trigger) is short
                fast = False  # psum-direct recip mis-shifts partitions on HW
                ou_sb, den0, recip, bcast, onorm = [], [], [], [], []
                if not fast:
                    for i in range(2):
                        ou_sb.append(npool.tile([65, SC], F32,
                                                tag=f"ou_sb{i}",
                                                name=f"ou_sb{i}"))
                        nc.vector.tensor_copy(ou_sb[i][:], ou[i][:])
                    for i in range(2):
                        den0.append(npool.tile([1, SC], F32, tag=f"den0{i}",
                                               name=f"den0{i}"))
                        nc.vector.tensor_copy(den0[i][0:1, :],
                                              ou_sb[i][64:65, :])
                for i in range(2):
                    recip.append(npool.tile([1, SC], F32, tag=f"recip{i}",
                                            name=f"recip{i}"))
                    nc.vector.reciprocal_approx_fast(
                        recip[i][0:1, :],
                        ou[i][64:65, :] if fast else den0[i][0:1, :],
                    )
                for i in range(2):
                    bcast.append(npool.tile([64, SC], F32, tag=f"bcast{i}",
                                            name=f"bcast{i}"))
                    nc.gpsimd.partition_broadcast(
                        bcast[i][:], recip[i][0:1, :]
                    )
                for i in range(2):
                    onorm.append(npool.tile([64, SC], BF16, tag=f"onorm{i}",
                                            name=f"onorm{i}"))
                    nc.vector.tensor_mul(
                        onorm[i][:],
                        ou[i][0:64, :] if fast else ou_sb[i][0:64, :],
                        bcast[i][:],
                    )
                    nc.sync.dma_start(
                        out=a2a_in[q // 2][4 * b + 2 * half + i][:, :],
                        in_=onorm[i][:],
                    )
                ascope.__exit__(None, None, None)
                return last_mm

            # ~3.5us of dummy matmuls during the initial DMA window so the
            # PE clock gate (HAM) is already open when real work issues.
            warm_ps = ps_proj.tile([64, SC], F32, tag="proj_ps", name="warm_ps")
            for w in range(5):
                nc.tensor.matmul(
                    warm_ps[:],
                    wqk_sb[:, 0, 0:64],
                    wqk_sb[:, 4 * (w % 2) : 4 * (w % 2) + 4, :],
                    start=True,
                    stop=True,
                )

            # ---- pair-0 prologue ----
            xT_t = {0: emit_xT(0)}
            qk_t = {0: alloc_qk(0)}
            v_t = {0: alloc_v(0)}
            emit_qk(xT_t[0], *qk_t[0], 0)
            emit_qk(xT_t[0], *qk_t[0], 1)

            last_mm = None
            for q in range(PAIRS):
                xT = xT_t.pop(q)
                qk_sb, qk2_sb = qk_t.pop(q)
                vT_sb, vt = v_t.pop(q)

                # boundary: first scores; pair 0's first v half is built
                # here, later pairs' in the previous pair's h1 fillers
                sc0 = emit_scores(qk_sb, qk2_sb, 0, 0)
                if q == 0:
                    v_mm(xT, vT_sb, 0)

                # NOTE: vt[t]'s writes must be EMITTED before attnv(t)
                # (iteration t) or Tile resolves the read against the
                # previous pair's tile -- so v_tp(t0) sits at slot <= t0.
                CLO, CHI = tuple(range(NDC // 2)), tuple(range(NDC // 2, NDC))
                f_h0 = {
                    0: [lambda: v_tp(vT_sb, vt, 0)],
                    1: [lambda: v_mm(xT, vT_sb, 1, CLO),
                        lambda: v_tp(vT_sb, vt, 2)],
                    2: [lambda: v_tp(vT_sb, vt, 4)],
                    3: [lambda: v_mm(xT, vT_sb, 1, CHI)],
                    4: [lambda: v_tp(vT_sb, vt, 6)],
                    7: [lambda: v_tp(vT_sb, vt, 8)],
                    8: [lambda: v_tp(vT_sb, vt, 10)],
                    9: [lambda: v_tp(vT_sb, vt, 12)],
                    10: [lambda: v_tp(vT_sb, vt, 14)],
                }
                if q + 1 < PAIRS:
                    f_h0[3].insert(
                        0, lambda: xT_t.__setitem__(q + 1, emit_xT(q + 1))
                    )
                    qk_t[q + 1] = alloc_qk(q + 1)
                    v_t[q + 1] = alloc_v(q + 1)
                    qkn, qk2n = qk_t[q + 1]
                    if q > 0:
                        f_h0[12] = [
                            lambda: emit_qk(xT_t[q + 1], qkn, qk2n, 0, CLO)
                        ]
                        f_h0[13] = [
                            lambda: emit_qk(xT_t[q + 1], qkn, qk2n, 0, CHI)
                        ]
                        f_h0[14] = [
                            lambda: emit_qk(xT_t[q + 1], qkn, qk2n, 1, CLO)
                        ]
                        f_h0[15] = [
                            lambda: emit_qk(xT_t[q + 1], qkn, qk2n, 1, CHI)
                        ]
                if q == 0:
                    # pair 0's own chunks 2/3 have in-half deadlines
                    # (kd tile t needs chunk t//4 by iteration t-1)
                    f_h0[2].append(lambda: emit_qk(xT, qk_sb, qk2_sb, 2, CLO))
                    f_h0[4].append(lambda: emit_qk(xT, qk_sb, qk2_sb, 2, CHI))
                    f_h0.setdefault(5, []).append(
                        lambda: emit_qk(xT, qk_sb, qk2_sb, 3, CLO)
                    )
                    f_h0.setdefault(6, []).append(
                        lambda: emit_qk(xT, qk_sb, qk2_sb, 3, CHI)
                    )
                if q == 1:
                    f_h0.setdefault(11, []).append(load_wo)
                run_half(q, 0, qk_sb, qk2_sb, vt, f_h0, sc0)

                sc0 = emit_scores(qk_sb, qk2_sb, 1, 0)
                f_h1 = {}
                if q + 1 < PAIRS:
                    qkn, qk2n = qk_t[q + 1]
                    vTn, _vtn = v_t[q + 1]
                    xTn_l = lambda: xT_t[q + 1]
                    for k in range(4):
                        n, cr = 2 + k // 2, (CLO if k % 2 == 0 else CHI)
                        f_h1[2 * k] = [
                            lambda n=n, cr=cr: emit_qk(
                                xT_t[q + 1], qkn, qk2n, n, cr
                            )
                        ]
                    if q == 0:
                        # pair-1 chunks 0/1 didn't fit in pair-0's h0
                        for k in range(4):
                            n, cr = k // 2, (CLO if k % 2 == 0 else CHI)
                            f_h1[8 + k] = [
                                lambda n=n, cr=cr: emit_qk(
                                    xT_t[q + 1], qkn, qk2n, n, cr
                                )
                            ]
                        f_h1[13] = [
                            lambda: v_mm(xT_t[q + 1], vTn, 0, CLO)
                        ]
                        f_h1[14] = [
                            lambda: v_mm(xT_t[q + 1], vTn, 0, CHI)
                        ]
                    else:
                        # next pair's first v half: its CASTs must land
                        # before the next h0's v_tp(0)
                        f_h1[9] = [
                            lambda: v_mm(xT_t[q + 1], vTn, 0, CLO)
                        ]
                        f_h1[11] = [
                            lambda: v_mm(xT_t[q + 1], vTn, 0, CHI)
                        ]
                last_mm = run_half(q, 1, qk_sb, qk2_sb, vt, f_h1, sc0)

                if q == 1:
                    # head-slot 0 of every core is done: redistribute
                    nc.gpsimd.collective_compute(
                        "AllToAll",
                        mybir.AluOpType.bypass,
                        replica_groups=[list(range(N_CORES))],
                        ins=[a2a_in[0].opt()],
                        outs=[a2a_out[0].opt()],
                    )
                    # even-head rows of the gathered activations
                    for c in range(NDC):
                        nc.sync.dma_start(
                            out=asb[0:64, c, :], in_=a2a_out[0][c][:, :]
                        )

            # ---- head-slot-1 redistribution + output projection tail ----
            nc.gpsimd.collective_compute(
                "AllToAll",
                mybir.AluOpType.bypass,
                replica_groups=[list(range(N_CORES))],
                ins=[a2a_in[1].opt()],
                outs=[a2a_out[1].opt()],
            )
            # keep the PE clock gate open across the AllToAll wait
            # (attention is done, so the sc slots are free for dummies)
            for w in range(60):
                warm2 = ps_sc.tile([64, SC], F32, tag="sc", name="warm2") \
                    if w % 20 == 0 else warm2
                nc.tensor.matmul(
                    warm2[:],
                    wqk_sb[:, 0, 0:64],
                    wqk_sb[:, 4 * (w % 2) : 4 * (w % 2) + 4, :],
                    start=True,
                    stop=True,
                )

            oscope = nc.named_scope("outproj")
            oscope.__enter__()
            # 6 m-tile accumulators run c-outer so the first matmuls start
            # as soon as the first odd-row chunk lands, pipelined with the
            # remaining asb DMAs; m=6,7 run m-outer afterwards.
            o_ps6 = [
                ps_sc.tile([128, SC], F32, tag="sc", name=f"o_ps{m}")
                for m in range(2)
            ] + [
                ps_proj.tile([128, SC], F32, tag="proj_ps", name=f"o_ps{m+2}")
                for m in range(2)
            ] + [
                ps_ou.tile([128, SC], F32, tag=f"ou{m}", name=f"o_ps{m+4}")
                for m in range(2)
            ]
            for c in range(NDC):
                nc.sync.dma_start(
                    out=asb[64:128, c, :], in_=a2a_out[1][c][:, :]
                )
                for m in range(6):
                    mm = nc.tensor.matmul(
                        o_ps6[m][:],
                        wo_sb[:, c, m, :],
                        asb[:, c, :],
                        start=(c == 0),
                        stop=(c == NDC - 1),
                    )
                    if last_mm is not None:
                        tile.add_dep_helper(
                            mm.ins, last_mm.ins, sync=False,
                            reason="outproj after attention",
                        )
                        last_mm = None
            for m in range(6):
                o_sb = opool.tile([128, SC], F32, tag="o_sb")
                nc.vector.tensor_copy(o_sb[:], o_ps6[m][:])
                nc.sync.dma_start(out=out[m][:, :], in_=o_sb[:])
            for m in (6, 7):
                o_ps = ps_sc.tile([128, SC], F32, tag="sc", name=f"o_ps{m}")
                for c in range(NDC):
                    nc.tensor.matmul(
                        o_ps[:],
                        wo_sb[:, c, m, :],
                        asb[:, c, :],
                        start=(c == 0),
                        stop=(c == NDC - 1),
                    )
                o_sb = opool.tile([128, SC], F32, tag="o_sb")
                nc.vector.tensor_copy(o_sb[:], o_ps[:])
                nc.sync.dma_start(out=out[m][:, :], in_=o_sb[:])
            oscope.__exit__(None, None, None)

    return nc


def _get_graph():
    global _GRAPH
    if _GRAPH is None:
        _GRAPH = _build_graph()
        if not _GRAPH.is_finalized():
            _GRAPH.finalize()
    return _GRAPH


def assemble(outs):
    # outs[r]: [8, 128, 512] f32 = out.T[:, 512r : 512r+512]
    full_t = np.concatenate(
        [np.asarray(o).reshape(D, SC) for o in outs], axis=1
    )  # [D, B*S]
    return np.ascontiguousarray(full_t.T).reshape(B, S, D)


def kernel(x, wq, wk, wv, wo):
    global LAST_RESULTS
    x = np.asarray(x, dtype=np.float32)
    wq = np.asarray(wq, dtype=np.float32)
    wk = np.asarray(wk, dtype=np.float32)
    wv = np.asarray(wv, dtype=np.float32)
    wo = np.asarray(wo, dtype=np.float32)

    bf16 = ml_dtypes.bfloat16
    # x transposed to [B, H, D, S] once (feeds matmuls as the moving operand)
    xt_all = np.ascontiguousarray(x.transpose(0, 1, 3, 2)).astype(bf16)
    wqk_t = np.ascontiguousarray(
        np.concatenate([wq, wk], axis=0).T
    ).astype(bf16)  # [D, 128]
    wv_t = np.ascontiguousarray(wv.T).astype(bf16)  # [D, 64]
    wv2_t = np.ascontiguousarray(
        np.concatenate([wv_t, wv_t], axis=1)
    )  # [D, 128] duplicated for col-tiled v projection
    wo_t = np.ascontiguousarray(wo.T).astype(bf16)  # [D, D], full per core

    in_maps = []
    for r in range(N_CORES):
        h0 = HPC * r
        # pair order: q = hl*B + b -> (b, h0+hl)
        xt_np = np.ascontiguousarray(
            xt_all[:, h0 : h0 + HPC]
            .transpose(1, 0, 2, 3)
            .reshape(PAIRS, D, S)
        )
        in_maps.append(
            {"xt": xt_np, "wqk": wqk_t, "wv2": wv2_t, "wo": wo_t}
        )

    nc = _get_graph()
    trace = bool(os.environ.get("BASS_TRACE"))
    if trace:
        try:  # tracing needs the axon NTFF hook; fall back cleanly
            from antenv.axon_hooks import get_axon_ntff_profile_hook  # noqa: F401
        except ImportError:
            trace = False
    tk = {}
    tc_env = os.environ.get("TRACE_CORES")
    if tc_env:
        tk["trace_cores"] = [int(c) for c in tc_env.split(",")]
    LAST_RESULTS = run_bass_kernel_spmd(
        nc, in_maps, core_ids=list(range(N_CORES)), trace=trace, **tk
    )
    outs = [LAST_RESULTS.results[r]["out"] for r in range(N_CORES)]
    return assemble(outs)
